# revision 8
# baseline (speedup 1.0000x reference)
"""Trainium2 Bass kernel for the ContractiveREN problem.

Strategy
--------
Data parallel over the batch: each of the 8 NeuronCores gets a 2048-row
shard of ``u_in``; all (small) parameter matrices are folded on the host
into four 128x128 fp16 matmul weights plus two per-partition fp32 bias
vectors.

Math
----
The reference computes (per batch row u, with x0 the initial state):
    w_i   = tanh((xc_i + ud_i + sum_{j<i} D11_ij w_j) / Lam_i)   (i = 0..127)
    y     = u @ Gu^T + w @ Gw^T + c0
where everything except the w-recurrence is affine in (u, w) and folds into
    Lhat = D11 / Lam[:,None],  xcl = xc/Lam,  UD = (D12/Lam) @ u^T
    Gu   = C2 @ inv(E) @ B2 + D22,  Gw = C2 @ inv(E) @ B1 + D21
    c0   = C2 @ inv(E) @ F @ x0
The strictly-lower-triangular recurrence is solved by fixed-point
iteration  W <- tanh(Lhat @ W + UD + xcl), which contracts the error by
~3.2x per sweep; 4 sweeps give rel err ~1e-3 against the fp32 reference
(the harness gate is 2e-2).

Implementation notes (all fp16 data / fp32 PSUM accumulation):
  * u is cast to fp16 on the host and loaded feature-major via the DMA
    XBAR transpose (dma_start_transpose) - no PE transposes at all.
  * Each 512-batch chunk owns one live PSUM bank holding
    UD + Lhat@W_k: pass k accumulates Lhat@(W_k - W_{k-1}) via matmul
    (the W-delta is a cheap all-fp16 DVE subtract), and every tanh
    applies xcl as the ACT bias, so no UDb tensor and no fp32 adds.
  * Output is computed feature-major (Gu@Ut + Gw@W accumulated in PSUM,
    + c0 as a DVE per-partition scalar add) and stored to DRAM
    feature-major as fp16; the host transposes/upcasts per core.
"""

import numpy as np

import concourse.bass as bass
import concourse.mybir as mybir
import concourse.tile as tile
from concourse import bacc
from concourse.bass_utils import run_bass_kernel_spmd

B = 16384
N_CORES = 8
BC = B // N_CORES  # 2048 batch rows per core
DIM_IN = 128
DIM_OUT = 128
DIM_X = 512
DIM_NL = 128
DIM_H = 2 * DIM_X + DIM_NL
EPS = 1e-3
ALPHA = 1.0
NCH = 4            # 512-column batch chunks (one PSUM bank each)
CW = BC // NCH     # 512
M_PASSES = 3       # delta-Jacobi passes after the seeded first sweep
F16 = mybir.dt.float16
F32 = mybir.dt.float32
TANH = mybir.ActivationFunctionType.Tanh

_BUILT = {}


def _build_nc():
    nc = bacc.Bacc("TRN2", target_bir_lowering=False, debug=False)
    u = nc.dram_tensor("u", [BC, DIM_IN], F16, kind="ExternalInput").ap()
    cst = nc.dram_tensor("cst", [128, 516], F16, kind="ExternalInput").ap()
    y = nc.dram_tensor("y", [DIM_OUT, BC], F16, kind="ExternalOutput").ap()

    u_g = u.rearrange("(g r) f -> g r f", g=NCH)

    IDENT = mybir.ActivationFunctionType.Identity
    u_h = u.rearrange("(g r) f -> g r f", g=2)  # two 1024-row load halves
    with tile.TileContext(nc) as tc:
        with (
            tc.tile_pool(name="const", bufs=1) as cpool,
            tc.tile_pool(name="big", bufs=1) as bpool,
            tc.tile_pool(name="w", bufs=2) as wpool,
            tc.tile_pool(name="d", bufs=2) as dpool,
            tc.tile_pool(name="yst", bufs=1) as ypool,
            tc.tile_pool(name="ps", bufs=1, space="PSUM") as pspool,
            tc.tile_pool(name="po", bufs=1, space="PSUM") as popool,
        ):
            # -- tanh table warm-up on ACT while input DMAs are in flight --
            tiny = cpool.tile([128, 1], F32, tag="tiny")
            nc.gpsimd.memset(tiny[:], 0.0)
            tiny2 = cpool.tile([128, 1], F32, tag="tiny2")

            # -- input DMAs. The XBAR transpose waits for every prior
            #    in-flight HWDGE DMA (shared unit), and the SP ring's first
            #    DMA waits on the ACT ring's first completion - so: u
            #    transposes go FIRST on the ACT ring, the constants ride
            #    the independent GpSimd software-DGE queue, and the SP
            #    ring is reserved for stores. --
            ut = bpool.tile([128, BC], F16, tag="ut")
            for g in range(2):
                nc.scalar.dma_start_transpose(
                    ut[:, g * 2 * CW:(g + 1) * 2 * CW], u_h[g]
                )
            cst_t = cpool.tile([128, 516], F16, tag="cst")
            nc.gpsimd.dma_start(cst_t[:], cst)
            nc.scalar.activation(tiny2[:], tiny[:], TANH)

            ltr = cst_t[:, 0:128]      # Lhat^T
            d12lt = cst_t[:, 128:256]  # (D12/Lam)^T
            gut = cst_t[:, 256:384]    # Gu^T
            gwt = cst_t[:, 384:512]    # Gw^T
            xcl = cst_t[:, 512:514].bitcast(F32)  # xc/Lam      [128,1] f32
            c0 = cst_t[:, 514:516].bitcast(F32)   # C2 Einv F x0 [128,1] f32

            ps = [
                pspool.tile([128, CW], F32, tag=f"ps{n}", name=f"ps{n}")
                for n in range(NCH)
            ]
            po = [
                popool.tile([128, CW], F32, tag=f"po{n}", name=f"po{n}")
                for n in range(NCH)
            ]

            # -- seed: ps = UD; Gu@Ut early-fills po while ACT runs tanh --
            for n in range(NCH):
                sl = slice(n * CW, (n + 1) * CW)
                nc.tensor.matmul(ps[n][:], d12lt, ut[:, sl],
                                 start=True, stop=False)
            for n in range(NCH):
                sl = slice(n * CW, (n + 1) * CW)
                nc.tensor.matmul(po[n][:], gut, ut[:, sl],
                                 start=True, stop=False)

            # -- W1 = tanh(ps + xcl) --
            w_cur = [None] * NCH
            for n in range(NCH):
                wt = wpool.tile([128, CW], F16, tag=f"w{n}", name=f"w{n}")
                nc.scalar.activation(wt[:], ps[n][:], TANH, bias=xcl)
                w_cur[n] = wt

            # -- delta-Jacobi passes: ps += Lhat @ (W_k - W_{k-1}) --
            w_prev = [None] * NCH
            for m in range(M_PASSES):
                last = m == M_PASSES - 1
                for n in range(NCH):
                    if m == 0:
                        dl = w_cur[n]  # W1 - 0
                    else:
                        dl = dpool.tile([128, CW], F16, tag=f"d{n}",
                                        name=f"d{n}")
                        nc.vector.tensor_sub(dl[:], w_cur[n][:], w_prev[n][:])
                    nc.tensor.matmul(ps[n][:], ltr, dl[:],
                                     start=False, stop=last)
                for n in range(NCH):
                    w_prev[n] = w_cur[n]
                    wt = wpool.tile([128, CW], F16, tag=f"w{n}", name=f"w{n}")
                    nc.scalar.activation(wt[:], ps[n][:], TANH, bias=xcl)
                    w_cur[n] = wt

            # -- output: po (= Gu@Ut) += Gw@W; yt = po + c0; store.
            #    c0-adds alternate DVE / ACT-Identity (same act table as
            #    tanh, no reload); store DMAs alternate the SP/ACT queues --
            for n in range(NCH):
                nc.tensor.matmul(po[n][:], gwt, w_cur[n][:],
                                 start=False, stop=True)
            for n in range(NCH):
                sl = slice(n * CW, (n + 1) * CW)
                yts = ypool.tile([128, CW], F16, tag=f"yt{n}", name=f"yt{n}")
                if n % 2 == 0:
                    nc.vector.tensor_scalar_add(yts[:], po[n][:], c0)
                else:
                    nc.scalar.activation(yts[:], po[n][:], IDENT, bias=c0)
                nc.sync.dma_start(y[:, sl], yts[:])
    nc.compile()
    return nc


def _derive_cst(X, Y, B2, C2, D21, D22, D12, x0):
    """Fold the contractive parameterization into kernel constants."""
    f = np.float32
    X = np.ascontiguousarray(X, f)
    H = (X.T @ X + EPS * np.eye(DIM_H, dtype=f)).astype(f)
    H11 = H[:DIM_X, :DIM_X]
    H21 = H[DIM_X:DIM_X + DIM_NL, :DIM_X]
    H22 = H[DIM_X:DIM_X + DIM_NL, DIM_X:DIM_X + DIM_NL]
    H31 = H[DIM_X + DIM_NL:, :DIM_X]
    H32 = H[DIM_X + DIM_NL:, DIM_X:DIM_X + DIM_NL]
    H33 = H[DIM_X + DIM_NL:, DIM_X + DIM_NL:]
    F = H31
    B1 = H32
    E = (0.5 * (H11 + ALPHA * H33 + Y - Y.T)).astype(f)
    Lam = (0.5 * np.diagonal(H22)).astype(f)
    D11 = (-np.tril(H22, k=-1)).astype(f)
    C1 = -H21

    Einv = np.linalg.inv(E).astype(f)
    x0v = np.asarray(x0, f)[0, 0, :]
    xc = (C1 @ x0v).astype(f)
    fx = (F @ x0v).astype(f)

    Lhat = (D11 / Lam[:, None]).astype(f)
    D12L = (np.asarray(D12, f) / Lam[:, None]).astype(f)
    CE = (np.asarray(C2, f) @ Einv).astype(f)
    Gu = (CE @ B2 + D22).astype(f)
    Gw = (CE @ B1 + D21).astype(f)
    xclam = (xc / Lam).astype(f)
    c0 = (CE @ fx).astype(f)

    cst = np.zeros((128, 516), np.float16)
    cst[:, 0:128] = Lhat.T.astype(np.float16)
    cst[:, 128:256] = D12L.T.astype(np.float16)
    cst[:, 256:384] = Gu.T.astype(np.float16)
    cst[:, 384:512] = Gw.T.astype(np.float16)
    cst[:, 512:514] = xclam.reshape(128, 1).view(np.float16)
    cst[:, 514:516] = c0.reshape(128, 1).view(np.float16)
    return cst


def _make_in_maps(u_in, X, Y, B2, C2, D21, D22, D12, x0):
    cst = _derive_cst(X, Y, B2, C2, D21, D22, D12, x0)
    u16 = np.ascontiguousarray(
        np.asarray(u_in, np.float32).reshape(B, DIM_IN).astype(np.float16)
    )
    return [
        {"u": u16[i * BC:(i + 1) * BC], "cst": cst}
        for i in range(N_CORES)
    ]


def kernel(u_in, X, Y, B2, C2, D21, D22, D12, x0):
    if "nc" not in _BUILT:
        _BUILT["nc"] = _build_nc()
    nc = _BUILT["nc"]
    in_maps = _make_in_maps(u_in, X, Y, B2, C2, D21, D22, D12, x0)
    res = run_bass_kernel_spmd(nc, in_maps, core_ids=list(range(N_CORES)))
    out = np.concatenate(
        [res.results[i]["y"].astype(np.float32).T for i in range(N_CORES)],
        axis=0,
    )
    return out.reshape(B, 1, DIM_OUT)


# revision 10
# speedup vs baseline: 1.0957x; 1.0957x over previous
"""Trainium2 Bass kernel for the ContractiveREN problem.

Strategy
--------
Data parallel over the batch: each of the 8 NeuronCores gets a 2048-row
shard of ``u_in``; all (small) parameter matrices are folded on the host
into four 128x128 fp16 matmul weights plus two per-partition fp32 bias
vectors.

Math
----
The reference computes (per batch row u, with x0 the initial state):
    w_i   = tanh((xc_i + ud_i + sum_{j<i} D11_ij w_j) / Lam_i)   (i = 0..127)
    y     = u @ Gu^T + w @ Gw^T + c0
where everything except the w-recurrence is affine in (u, w) and folds into
    Lhat = D11 / Lam[:,None],  xcl = xc/Lam,  UD = (D12/Lam) @ u^T
    Gu   = C2 @ inv(E) @ B2 + D22,  Gw = C2 @ inv(E) @ B1 + D21
    c0   = C2 @ inv(E) @ F @ x0
The strictly-lower-triangular recurrence is solved by fixed-point
iteration  W <- tanh(Lhat @ W + UD + xcl), which contracts the error by
~3.2x per sweep; 3 sweeps give rel err ~3.4e-3 against the fp32
reference (the harness gate is 2e-2; measured hw error matches the host
fp16 simulation exactly).

Implementation notes (all fp16 data / fp32 PSUM accumulation):
  * The whole input - u AND the folded constants (pre-transposed on the
    host) - rides ONE DRAM tensor loaded feature-major via XBAR DMA
    transposes (dma_start_transpose). The scheduler serializes any DMA
    against in-flight XBAR transposes, so there are no other input DMAs,
    and constants are interleaved early in the stream (seed weights
    first, pass/output weights after the first u chunk).
  * Each 512-batch chunk owns one live PSUM bank holding
    UD + Lhat@W_k: pass k accumulates Lhat@(W_k - W_{k-1}) via matmul
    (the W-delta is a cheap all-fp16 DVE subtract), and every tanh
    applies xcl as the ACT bias, so no UDb tensor and no fp32 adds.
  * Output is computed feature-major (Gu@Ut early + Gw@W accumulated in
    PSUM, + c0 via DVE tensor-scalar / ACT-Identity-bias alternating)
    and stored to DRAM feature-major as fp16; the host transposes and
    upcasts per core.
"""

import numpy as np

import concourse.bass as bass
import concourse.mybir as mybir
import concourse.tile as tile
from concourse import bacc
from concourse.bass_utils import run_bass_kernel_spmd

B = 16384
N_CORES = 8
BC = B // N_CORES  # 2048 batch rows per core
DIM_IN = 128
DIM_OUT = 128
DIM_X = 512
DIM_NL = 128
DIM_H = 2 * DIM_X + DIM_NL
EPS = 1e-3
ALPHA = 1.0
NCH = 4            # 512-column batch chunks (one PSUM bank each)
CW = BC // NCH     # 512
M_PASSES = 2       # delta-Jacobi passes after the seeded first sweep
CA = 144           # cstA rows: d12lt(128) + xcl(2) + c0(2) + pad(12)
CB = 384           # cstB rows: ltr(128) + gut(128) + gwt(128)
NROW = CA + CB + BC  # 2576 rows in the merged input tensor
F16 = mybir.dt.float16
F32 = mybir.dt.float32
TANH = mybir.ActivationFunctionType.Tanh
IDENT = mybir.ActivationFunctionType.Identity

_BUILT = {}


def _build_nc():
    nc = bacc.Bacc("TRN2", target_bir_lowering=False, debug=False)
    ub = nc.dram_tensor("ub", [NROW, DIM_IN], F16, kind="ExternalInput").ap()
    y = nc.dram_tensor("y", [DIM_OUT, BC], F16, kind="ExternalOutput").ap()

    # row layout of ub: [cstA (144) | u chunk0 (512) | cstB (384) |
    #                    u chunk1..3 (3*512)]
    r_ca = ub[0:CA]
    r_u = [None] * NCH
    r_u[0] = ub[CA:CA + CW]
    r_cb = ub[CA + CW:CA + CW + CB]
    for n in range(1, NCH):
        base = CA + CB + n * CW
        r_u[n] = ub[base:base + CW]

    with tile.TileContext(nc) as tc:
        with (
            tc.tile_pool(name="const", bufs=1) as cpool,
            tc.tile_pool(name="big", bufs=1) as bpool,
            tc.tile_pool(name="w", bufs=2) as wpool,
            tc.tile_pool(name="d", bufs=2) as dpool,
            tc.tile_pool(name="yst", bufs=1) as ypool,
            tc.tile_pool(name="ps", bufs=1, space="PSUM") as pspool,
            tc.tile_pool(name="po", bufs=1, space="PSUM") as popool,
        ):
            # -- tanh table warm-up on ACT while input DMAs are in flight --
            tiny = cpool.tile([128, 1], F32, tag="tiny")
            nc.gpsimd.memset(tiny[:], 0.0)
            tiny2 = cpool.tile([128, 1], F32, tag="tiny2")
            nc.scalar.activation(tiny2[:], tiny[:], TANH)

            # -- input XBAR transposes, ALL on the SP ring: the XBAR is a
            #    single shared unit and transposes on both rings run
            #    concurrently and corrupt each other (the scheduler only
            #    serializes transposes against regular DMAs). One ring =
            #    hardware FIFO = safe, and it leaves the ACT queue free
            #    for the tanh stream. Stores ride the same ring later. --
            ctA = cpool.tile([128, CA], F16, tag="ctA")
            ctB = cpool.tile([128, CB], F16, tag="ctB")
            ut = bpool.tile([128, BC], F16, tag="ut")
            nc.sync.dma_start_transpose(ctA[:], r_ca)
            nc.sync.dma_start_transpose(ut[:, 0:CW], r_u[0])
            nc.sync.dma_start_transpose(ctB[:], r_cb)
            for n in range(1, NCH):
                nc.sync.dma_start_transpose(
                    ut[:, n * CW:(n + 1) * CW], r_u[n]
                )

            d12lt = ctA[:, 0:128]                 # (D12/Lam)^T
            xcl = ctA[:, 128:130].bitcast(F32)    # xc/Lam       [128,1] f32
            c0 = ctA[:, 130:132].bitcast(F32)     # C2 Einv F x0 [128,1] f32
            ltr = ctB[:, 0:128]                   # Lhat^T
            gut = ctB[:, 128:256]                 # Gu^T
            gwt = ctB[:, 256:384]                 # Gw^T

            ps = [
                pspool.tile([128, CW], F32, tag=f"ps{n}", name=f"ps{n}")
                for n in range(NCH)
            ]
            po = [
                popool.tile([128, CW], F32, tag=f"po{n}", name=f"po{n}")
                for n in range(NCH)
            ]

            w_cur = [None] * NCH
            w_prev = [None] * NCH

            def seed_mm(n):
                sl = slice(n * CW, (n + 1) * CW)
                nc.tensor.matmul(ps[n][:], d12lt, ut[:, sl],
                                 start=True, stop=False)

            def gu_mm(n):
                sl = slice(n * CW, (n + 1) * CW)
                nc.tensor.matmul(po[n][:], gut, ut[:, sl],
                                 start=True, stop=False)

            def seed_tanh(n):
                wt = wpool.tile([128, CW], F16, tag=f"w{n}", name=f"w{n}")
                nc.scalar.activation(wt[:], ps[n][:], TANH, bias=xcl)
                w_cur[n] = wt

            def pass_mm(n, m):
                # ps += Lhat @ (W_k - W_{k-1}); the delta for the first
                # pass is W1 itself.
                if m == 0:
                    dl = w_cur[n]
                else:
                    dl = dpool.tile([128, CW], F16, tag=f"d{n}", name=f"d{n}")
                    nc.vector.tensor_sub(dl[:], w_cur[n][:], w_prev[n][:])
                nc.tensor.matmul(ps[n][:], ltr, dl[:],
                                 start=False, stop=(m == M_PASSES - 1))

            def pass_tanh(n):
                w_prev[n] = w_cur[n]
                wt = wpool.tile([128, CW], F16, tag=f"w{n}", name=f"w{n}")
                nc.scalar.activation(wt[:], ps[n][:], TANH, bias=xcl)
                w_cur[n] = wt

            def gw_mm(n):
                nc.tensor.matmul(po[n][:], gwt, w_cur[n][:],
                                 start=False, stop=True)

            def store(n):
                sl = slice(n * CW, (n + 1) * CW)
                yts = ypool.tile([128, CW], F16, tag=f"yt{n}", name=f"yt{n}")
                if n % 2 == 0:
                    nc.vector.tensor_scalar_add(yts[:], po[n][:], c0)
                else:
                    nc.scalar.activation(yts[:], po[n][:], IDENT, bias=c0)
                nc.sync.dma_start(y[:, sl], yts[:])

            # Emission order = per-engine execution order; PE matmuls are
            # laid out in expected data-arrival order (u chunks land
            # serially from the XBAR) to avoid head-of-line stalls.
            seed_mm(0); gu_mm(0)
            seed_tanh(0)
            pass_mm(0, 0)
            seed_mm(1); gu_mm(1)
            pass_tanh(0)
            seed_tanh(1)
            pass_mm(1, 0); pass_mm(0, 1)
            pass_tanh(1)
            pass_tanh(0)
            seed_mm(2); gu_mm(2)
            gw_mm(0)
            seed_tanh(2)
            store(0)
            pass_mm(2, 0); pass_mm(1, 1)
            pass_tanh(2)
            pass_tanh(1)
            seed_mm(3); gu_mm(3)
            gw_mm(1)
            seed_tanh(3)
            store(1)
            pass_mm(3, 0); pass_mm(2, 1)
            pass_tanh(3)
            pass_tanh(2)
            gw_mm(2)
            store(2)
            pass_mm(3, 1)
            pass_tanh(3)
            gw_mm(3)
            store(3)
    nc.compile()
    return nc


def _derive_consts(X, Y, B2, C2, D21, D22, D12, x0):
    """Fold the contractive parameterization into kernel constants,
    returned as the (row-major, pre-transposed) cstA/cstB DRAM blocks."""
    f = np.float32
    X = np.ascontiguousarray(X, f)
    H = (X.T @ X + EPS * np.eye(DIM_H, dtype=f)).astype(f)
    H11 = H[:DIM_X, :DIM_X]
    H21 = H[DIM_X:DIM_X + DIM_NL, :DIM_X]
    H22 = H[DIM_X:DIM_X + DIM_NL, DIM_X:DIM_X + DIM_NL]
    H31 = H[DIM_X + DIM_NL:, :DIM_X]
    H32 = H[DIM_X + DIM_NL:, DIM_X:DIM_X + DIM_NL]
    H33 = H[DIM_X + DIM_NL:, DIM_X + DIM_NL:]
    F = H31
    B1 = H32
    E = (0.5 * (H11 + ALPHA * H33 + Y - Y.T)).astype(f)
    Lam = (0.5 * np.diagonal(H22)).astype(f)
    D11 = (-np.tril(H22, k=-1)).astype(f)
    C1 = -H21

    Einv = np.linalg.inv(E).astype(f)
    x0v = np.asarray(x0, f)[0, 0, :]
    xc = (C1 @ x0v).astype(f)
    fx = (F @ x0v).astype(f)

    Lhat = (D11 / Lam[:, None]).astype(f)
    D12L = (np.asarray(D12, f) / Lam[:, None]).astype(f)
    CE = (np.asarray(C2, f) @ Einv).astype(f)
    Gu = (CE @ B2 + D22).astype(f)
    Gw = (CE @ B1 + D21).astype(f)
    xclam = (xc / Lam).astype(f)
    c0 = (CE @ fx).astype(f)

    h = np.float16
    cstA = np.zeros((128, CA), h)
    cstA[:, 0:128] = D12L.T.astype(h)
    cstA[:, 128:130] = xclam.reshape(128, 1).view(h)
    cstA[:, 130:132] = c0.reshape(128, 1).view(h)
    cstB = np.zeros((128, CB), h)
    cstB[:, 0:128] = Lhat.T.astype(h)
    cstB[:, 128:256] = Gu.T.astype(h)
    cstB[:, 256:384] = Gw.T.astype(h)
    return np.ascontiguousarray(cstA.T), np.ascontiguousarray(cstB.T)


def _make_in_maps(u_in, X, Y, B2, C2, D21, D22, D12, x0):
    cstAT, cstBT = _derive_consts(X, Y, B2, C2, D21, D22, D12, x0)
    u16 = np.asarray(u_in, np.float32).reshape(B, DIM_IN).astype(np.float16)
    maps = []
    for i in range(N_CORES):
        uc = u16[i * BC:(i + 1) * BC]
        ub = np.concatenate(
            [cstAT, uc[0:CW], cstBT, uc[CW:]], axis=0
        )
        maps.append({"ub": np.ascontiguousarray(ub)})
    return maps


def kernel(u_in, X, Y, B2, C2, D21, D22, D12, x0):
    if "nc" not in _BUILT:
        _BUILT["nc"] = _build_nc()
    nc = _BUILT["nc"]
    in_maps = _make_in_maps(u_in, X, Y, B2, C2, D21, D22, D12, x0)
    res = run_bass_kernel_spmd(nc, in_maps, core_ids=list(range(N_CORES)))
    out = np.concatenate(
        [res.results[i]["y"].astype(np.float32).T for i in range(N_CORES)],
        axis=0,
    )
    return out.reshape(B, 1, DIM_OUT)


# revision 11
# speedup vs baseline: 1.1126x; 1.0154x over previous
"""Trainium2 Bass kernel for the ContractiveREN problem.

Strategy
--------
Data parallel over the batch: each of the 8 NeuronCores gets a 2048-row
shard of ``u_in``; all (small) parameter matrices are folded on the host
into four 128x128 fp16 matmul weights plus two per-partition fp32 bias
vectors.

Math
----
The reference computes (per batch row u, with x0 the initial state):
    w_i   = tanh((xc_i + ud_i + sum_{j<i} D11_ij w_j) / Lam_i)   (i = 0..127)
    y     = u @ Gu^T + w @ Gw^T + c0
where everything except the w-recurrence is affine in (u, w) and folds into
    Lhat = D11 / Lam[:,None],  xcl = xc/Lam,  UD = (D12/Lam) @ u^T
    Gu   = C2 @ inv(E) @ B2 + D22,  Gw = C2 @ inv(E) @ B1 + D21
    c0   = C2 @ inv(E) @ F @ x0
The strictly-lower-triangular recurrence is solved by fixed-point
iteration  W <- tanh(Lhat @ W + UD + xcl), which contracts the error by
~3.2x per sweep; 3 sweeps give rel err ~3.4e-3 against the fp32
reference (the harness gate is 2e-2; measured hw error matches the host
fp16 simulation exactly).

Implementation notes (all fp16 data / fp32 PSUM accumulation):
  * Input loads avoid both row-granular DMA (descriptor-rate bound) and
    the XBAR DMA transpose (slow and serializing): the host pre-scatters
    u and the constants so ONE big-descriptor DMA per region lands them
    in SBUF as 128x128 blocks, which PE block-transposes (identity
    generated on device via affine_select) and DVE copies reassemble
    feature-major. Total DMA: 2 instructions with 4KB/1KB descriptors.
  * Each 512-batch chunk owns one live PSUM bank holding
    UD + Lhat@W_k: pass k accumulates Lhat@(W_k - W_{k-1}) via matmul
    (the W-delta is a cheap all-fp16 DVE subtract), and every tanh
    applies xcl as the ACT bias, so no UDb tensor and no fp32 adds.
  * Output is computed feature-major at each chunk's tail (Gu@Ut +
    Gw@W accumulated into the chunk's freed ps bank, + c0 via DVE
    tensor-scalar / ACT-Identity-bias alternating) and stored to DRAM
    feature-major as fp16; the host transposes and upcasts per core.
"""

import numpy as np

import concourse.bass as bass
import concourse.mybir as mybir
import concourse.tile as tile
from concourse import bacc
from concourse.bass_utils import run_bass_kernel_spmd

B = 16384
N_CORES = 8
BC = B // N_CORES  # 2048 batch rows per core
DIM_IN = 128
DIM_OUT = 128
DIM_X = 512
DIM_NL = 128
DIM_H = 2 * DIM_X + DIM_NL
EPS = 1e-3
ALPHA = 1.0
NCH = 4            # 512-column batch chunks (one PSUM bank each)
CW = BC // NCH     # 512
M_PASSES = 2       # delta-Jacobi passes after the seeded first sweep
CROWS = 512        # constant region rows (4 x 128 matrices)
UROWS = 17 * 128   # u region rows: 16 batch blocks + 1 bias/pad block
F16 = mybir.dt.float16
F32 = mybir.dt.float32
TANH = mybir.ActivationFunctionType.Tanh
IDENT = mybir.ActivationFunctionType.Identity
EQ = mybir.AluOpType.is_equal

_BUILT = {}


def _build_nc():
    nc = bacc.Bacc("TRN2", target_bir_lowering=False, debug=False)
    ub = nc.dram_tensor(
        "ub", [CROWS + UROWS, DIM_IN], F16, kind="ExternalInput"
    ).ap()
    y = nc.dram_tensor("y", [DIM_OUT, BC], F16, kind="ExternalOutput").ap()

    # big-descriptor views: partition p reads R consecutive DRAM rows
    r_cst = ub[0:CROWS].rearrange("(p r) f -> p (r f)", p=128)       # 1KB/desc
    r_u = ub[CROWS:].rearrange("(p r) f -> p (r f)", p=128)          # 4.25KB

    with tile.TileContext(nc) as tc:
        with (
            tc.tile_pool(name="const", bufs=1) as cpool,
            tc.tile_pool(name="big", bufs=1) as bpool,
            tc.tile_pool(name="w", bufs=2) as wpool,
            tc.tile_pool(name="d", bufs=2) as dpool,
            tc.tile_pool(name="yst", bufs=1) as ypool,
            tc.tile_pool(name="ps", bufs=1, space="PSUM") as pspool,
            tc.tile_pool(name="tst", bufs=2, space="PSUM") as tpool,
        ):
            # -- on-device identity (for PE block transposes) + tanh
            #    table warm-up, all during the DMA-in window --
            ones = cpool.tile([128, 128], F16, tag="ones")
            nc.gpsimd.memset(ones[:], 1.0)
            ident = cpool.tile([128, 128], F16, tag="ident")
            nc.gpsimd.affine_select(
                ident[:], ones[:], pattern=[[-1, 128]], compare_op=EQ,
                fill=0.0, base=0, channel_multiplier=1,
            )
            tiny = cpool.tile([128, 1], F32, tag="tiny")
            nc.gpsimd.memset(tiny[:], 0.0)
            tiny2 = cpool.tile([128, 1], F32, tag="tiny2")
            nc.scalar.activation(tiny2[:], tiny[:], TANH)

            # -- the two input DMAs (SP ring; stores ride behind later) --
            cstage = cpool.tile([128, CROWS], F16, tag="cstage")
            nc.sync.dma_start(cstage[:], r_cst)
            ustage = bpool.tile([128, UROWS], F16, tag="ustage")
            nc.sync.dma_start(ustage[:], r_u)

            # -- constants: PE block-transpose + stride-4 DVE reassembly.
            #    C[:, d] = ub[d, :]^T, so matrix j sits at cols 128j. --
            C = cpool.tile([128, CROWS], F16, tag="C")
            C_r = C.rearrange("p (d r) -> r p d", r=4)
            tpc = tpool.tile([128, CROWS], F16, tag="tst", name="tpc")
            for r in range(4):
                nc.tensor.transpose(
                    tpc[:, r * 128:(r + 1) * 128],
                    cstage[:, r * 128:(r + 1) * 128], ident[:],
                )
            for r in range(4):
                nc.vector.tensor_copy(C_r[r], tpc[:, r * 128:(r + 1) * 128])

            d12lt = C[:, 0:128]    # (D12/Lam)^T
            ltr = C[:, 128:256]    # Lhat^T
            gut = C[:, 256:384]    # Gu^T
            gwt = C[:, 384:512]    # Gw^T

            # -- bias vectors ride block 16 of the u region --
            btile = cpool.tile([128, 4], F16, tag="btile")
            tpb = tpool.tile([128, CROWS], F16, tag="tst", name="tpb")
            nc.tensor.transpose(
                tpb[:, 0:128], ustage[:, 16 * 128:17 * 128], ident[:]
            )
            nc.vector.tensor_copy(btile[:], tpb[:, 0:4])
            xcl = btile[:, 0:2].bitcast(F32)  # xc/Lam       [128,1] f32
            c0 = btile[:, 2:4].bitcast(F32)   # C2 Einv F x0 [128,1] f32

            ps = [
                pspool.tile([128, CW], F32, tag=f"ps{n}", name=f"ps{n}")
                for n in range(NCH)
            ]
            ut = bpool.tile([128, BC], F16, tag="ut")

            w_cur = [None] * NCH
            w_prev = [None] * NCH

            # -- u chunks: transpose 4 blocks -> one packed copy -> seed --
            for n in range(NCH):
                tpu = tpool.tile([128, CROWS], F16, tag="tst", name=f"tpu{n}")
                for k in range(4):
                    b = 4 * n + k
                    nc.tensor.transpose(
                        tpu[:, k * 128:(k + 1) * 128],
                        ustage[:, b * 128:(b + 1) * 128], ident[:],
                    )
                sl = slice(n * CW, (n + 1) * CW)
                nc.vector.tensor_copy(ut[:, sl], tpu[:])
                nc.tensor.matmul(ps[n][:], d12lt, ut[:, sl],
                                 start=True, stop=False)
                wt = wpool.tile([128, CW], F16, tag=f"w{n}", name=f"w{n}")
                nc.scalar.activation(wt[:], ps[n][:], TANH, bias=xcl)
                w_cur[n] = wt

            # -- delta-Jacobi passes: ps += Lhat @ (W_k - W_{k-1}) --
            for m in range(M_PASSES):
                last = m == M_PASSES - 1
                for n in range(NCH):
                    if m == 0:
                        dl = w_cur[n]  # W1 - 0
                    else:
                        dl = dpool.tile([128, CW], F16, tag=f"d{n}",
                                        name=f"d{n}")
                        nc.vector.tensor_sub(dl[:], w_cur[n][:], w_prev[n][:])
                    nc.tensor.matmul(ps[n][:], ltr, dl[:],
                                     start=False, stop=last)
                for n in range(NCH):
                    w_prev[n] = w_cur[n]
                    wt = wpool.tile([128, CW], F16, tag=f"w{n}", name=f"w{n}")
                    nc.scalar.activation(wt[:], ps[n][:], TANH, bias=xcl)
                    w_cur[n] = wt

            # -- per-chunk tail: po (reusing the chunk's freed ps bank) =
            #    Gu@Ut + Gw@W; yt = po + c0; store. c0-adds alternate
            #    DVE / ACT-Identity (same act table as tanh, no reload) --
            for n in range(NCH):
                sl = slice(n * CW, (n + 1) * CW)
                po = pspool.tile([128, CW], F32, tag=f"ps{n}", name=f"po{n}")
                nc.tensor.matmul(po[:], gut, ut[:, sl], start=True, stop=False)
                nc.tensor.matmul(po[:], gwt, w_cur[n][:],
                                 start=False, stop=True)
                yts = ypool.tile([128, CW], F16, tag=f"yt{n}", name=f"yt{n}")
                if n % 2 == 0:
                    nc.vector.tensor_scalar_add(yts[:], po[:], c0)
                else:
                    nc.scalar.activation(yts[:], po[:], IDENT, bias=c0)
                nc.sync.dma_start(y[:, sl], yts[:])
    nc.compile()
    return nc


def _derive_consts(X, Y, B2, C2, D21, D22, D12, x0):
    """Fold the contractive parameterization into kernel constants.
    Returns (cst_rows [512,128] f16, bias_rows [4,128] f16)."""
    f = np.float32
    X = np.ascontiguousarray(X, f)
    H = (X.T @ X + EPS * np.eye(DIM_H, dtype=f)).astype(f)
    H11 = H[:DIM_X, :DIM_X]
    H21 = H[DIM_X:DIM_X + DIM_NL, :DIM_X]
    H22 = H[DIM_X:DIM_X + DIM_NL, DIM_X:DIM_X + DIM_NL]
    H31 = H[DIM_X + DIM_NL:, :DIM_X]
    H32 = H[DIM_X + DIM_NL:, DIM_X:DIM_X + DIM_NL]
    H33 = H[DIM_X + DIM_NL:, DIM_X + DIM_NL:]
    F = H31
    B1 = H32
    E = (0.5 * (H11 + ALPHA * H33 + Y - Y.T)).astype(f)
    Lam = (0.5 * np.diagonal(H22)).astype(f)
    D11 = (-np.tril(H22, k=-1)).astype(f)
    C1 = -H21

    Einv = np.linalg.inv(E).astype(f)
    x0v = np.asarray(x0, f)[0, 0, :]
    xc = (C1 @ x0v).astype(f)
    fx = (F @ x0v).astype(f)

    Lhat = (D11 / Lam[:, None]).astype(f)
    D12L = (np.asarray(D12, f) / Lam[:, None]).astype(f)
    CE = (np.asarray(C2, f) @ Einv).astype(f)
    Gu = (CE @ B2 + D22).astype(f)
    Gw = (CE @ B1 + D21).astype(f)
    xclam = (xc / Lam).astype(f)
    c0 = (CE @ fx).astype(f)

    h = np.float16
    cst = np.concatenate(
        [D12L.astype(h), Lhat.astype(h), Gu.astype(h), Gw.astype(h)], axis=0
    )
    bias = np.zeros((4, 128), h)
    xb = xclam.view(np.uint32)
    cb = c0.view(np.uint32)
    bias[0] = (xb & 0xFFFF).astype(np.uint16).view(h)   # xcl low half
    bias[1] = (xb >> 16).astype(np.uint16).view(h)      # xcl high half
    bias[2] = (cb & 0xFFFF).astype(np.uint16).view(h)
    bias[3] = (cb >> 16).astype(np.uint16).view(h)
    return cst, bias


def _make_in_maps(u_in, X, Y, B2, C2, D21, D22, D12, x0):
    cst, bias = _derive_consts(X, Y, B2, C2, D21, D22, D12, x0)
    u16 = np.asarray(u_in, np.float32).reshape(B, DIM_IN).astype(np.float16)
    maps = []
    for i in range(N_CORES):
        uc = u16[i * BC:(i + 1) * BC]
        # partition p holds rows {17p + r}; block r must be batch rows
        # {128r + p}, block 16 the bias rows.
        S = np.zeros((128, 17, DIM_IN), np.float16)
        S[:, 0:16, :] = uc.reshape(16, 128, DIM_IN).transpose(1, 0, 2)
        S[0:4, 16, :] = bias
        ubuf = np.concatenate([cst, S.reshape(UROWS, DIM_IN)], axis=0)
        maps.append({"ub": np.ascontiguousarray(ubuf)})
    return maps


def kernel(u_in, X, Y, B2, C2, D21, D22, D12, x0):
    if "nc" not in _BUILT:
        _BUILT["nc"] = _build_nc()
    nc = _BUILT["nc"]
    in_maps = _make_in_maps(u_in, X, Y, B2, C2, D21, D22, D12, x0)
    res = run_bass_kernel_spmd(nc, in_maps, core_ids=list(range(N_CORES)))
    out = np.concatenate(
        [res.results[i]["y"].astype(np.float32).T for i in range(N_CORES)],
        axis=0,
    )
    return out.reshape(B, 1, DIM_OUT)


# revision 12
# speedup vs baseline: 1.1671x; 1.0490x over previous
"""Trainium2 Bass kernel for the ContractiveREN problem.

Strategy
--------
Data parallel over the batch: each of the 8 NeuronCores gets a 2048-row
shard of ``u_in``; all (small) parameter matrices are folded on the host
into four 128x128 fp16 matmul weights plus two per-partition fp32 bias
vectors.

Math
----
The reference computes (per batch row u, with x0 the initial state):
    w_i   = tanh((xc_i + ud_i + sum_{j<i} D11_ij w_j) / Lam_i)   (i = 0..127)
    y     = u @ Gu^T + w @ Gw^T + c0
where everything except the w-recurrence is affine in (u, w) and folds into
    Lhat = D11 / Lam[:,None],  xcl = xc/Lam,  UD = (D12/Lam) @ u^T
    Gu   = C2 @ inv(E) @ B2 + D22,  Gw = C2 @ inv(E) @ B1 + D21
    c0   = C2 @ inv(E) @ F @ x0
The strictly-lower-triangular recurrence is solved by fixed-point
iteration  W <- tanh(Lhat @ W + UD + xcl), which contracts the error by
~3.2x per sweep; 3 sweeps give rel err ~3.4e-3 against the fp32
reference (the harness gate is 2e-2; measured hw error matches the host
fp16 simulation exactly).

Implementation notes (all fp16 data / fp32 PSUM accumulation):
  * Input loads avoid both row-granular DMA (descriptor-rate bound) and
    the XBAR DMA transpose (slow, and serializing against every other
    DMA): the host pre-scatters u and the constants so a handful of
    big-descriptor DMAs land them in SBUF as 128x128 blocks, which PE
    block-transposes (identity generated on device via affine_select)
    and cheap contiguous DVE copies reassemble feature-major. Each
    512-batch chunk rides its own DMA so the pipeline starts on the
    first chunk while the rest are in flight.
  * Each 512-batch chunk owns one live PSUM bank holding
    UD + Lhat@W_k: pass k accumulates Lhat@(W_k - W_{k-1}) via matmul
    (the W-delta is a cheap all-fp16 DVE subtract), and every tanh
    applies xcl as the ACT bias, so no UDb tensor and no fp32 adds.
  * Output is computed feature-major at each chunk's tail (Gu@Ut +
    Gw@W accumulated into the chunk's freed ps bank, + c0 via DVE
    tensor-scalar / ACT-Identity-bias alternating) and stored to DRAM
    feature-major as fp16; the host transposes and upcasts per core.
"""

import numpy as np

import concourse.bass as bass
import concourse.mybir as mybir
import concourse.tile as tile
from concourse import bacc
from concourse.bass_utils import run_bass_kernel_spmd

B = 16384
N_CORES = 8
BC = B // N_CORES  # 2048 batch rows per core
DIM_IN = 128
DIM_OUT = 128
DIM_X = 512
DIM_NL = 128
DIM_H = 2 * DIM_X + DIM_NL
EPS = 1e-3
ALPHA = 1.0
NCH = 4            # 512-column batch chunks (one PSUM bank each)
CW = BC // NCH     # 512
M_PASSES = 2       # delta-Jacobi passes after the seeded first sweep
CROWS = 5 * 128    # constant region: D12L | Lhat | Gu | Gw | bias blocks
F16 = mybir.dt.float16
F32 = mybir.dt.float32
TANH = mybir.ActivationFunctionType.Tanh
IDENT = mybir.ActivationFunctionType.Identity
EQ = mybir.AluOpType.is_equal

_BUILT = {}


def _build_nc():
    nc = bacc.Bacc("TRN2", target_bir_lowering=False, debug=False)
    ub = nc.dram_tensor(
        "ub", [CROWS + BC, DIM_IN], F16, kind="ExternalInput"
    ).ap()
    y = nc.dram_tensor("y", [DIM_OUT, BC], F16, kind="ExternalOutput").ap()

    # big-descriptor views: partition p reads R consecutive DRAM rows,
    # giving SBUF col (r*128 + f) = DRAM row (R*p + r), feature f.
    r_cst = ub[0:CROWS].rearrange("(p r) f -> p (r f)", p=128)
    r_u = [
        ub[CROWS + g * CW:CROWS + (g + 1) * CW].rearrange(
            "(p r) f -> p (r f)", p=128
        )
        for g in range(NCH)
    ]

    with tile.TileContext(nc) as tc:
        with (
            tc.tile_pool(name="const", bufs=1) as cpool,
            tc.tile_pool(name="big", bufs=1) as bpool,
            tc.tile_pool(name="w", bufs=2) as wpool,
            tc.tile_pool(name="d", bufs=2) as dpool,
            tc.tile_pool(name="yst", bufs=1) as ypool,
            tc.tile_pool(name="ps", bufs=1, space="PSUM") as pspool,
            tc.tile_pool(name="tst", bufs=2, space="PSUM") as tpool,
        ):
            # -- on-device identity (for PE block transposes) + tanh
            #    table warm-up, all during the DMA-in window --
            ones = cpool.tile([128, 128], F16, tag="ones")
            nc.gpsimd.memset(ones[:], 1.0)
            ident = cpool.tile([128, 128], F16, tag="ident")
            nc.gpsimd.affine_select(
                ident[:], ones[:], pattern=[[-1, 128]], compare_op=EQ,
                fill=0.0, base=0, channel_multiplier=1,
            )
            tiny = cpool.tile([128, 1], F32, tag="tiny")
            nc.gpsimd.memset(tiny[:], 0.0)
            tiny2 = cpool.tile([128, 1], F32, tag="tiny2")
            nc.scalar.activation(tiny2[:], tiny[:], TANH)

            # -- input DMAs (SP ring; stores ride behind later) --
            cstage = cpool.tile([128, CROWS], F16, tag="cstage")
            nc.sync.dma_start(cstage[:], r_cst)
            ustage = bpool.tile([128, BC], F16, tag="ustage")
            for g in range(NCH):
                nc.sync.dma_start(ustage[:, g * CW:(g + 1) * CW], r_u[g])

            cblk = lambda j: cstage[:, j * 128:(j + 1) * 128]
            d12lt = cpool.tile([128, 128], F16, tag="d12lt")
            ltr = cpool.tile([128, 128], F16, tag="ltr")
            gut = cpool.tile([128, 128], F16, tag="gut")
            gwt = cpool.tile([128, 128], F16, tag="gwt")
            btile = cpool.tile([128, 4], F16, tag="btile")
            xcl = btile[:, 0:2].bitcast(F32)  # xc/Lam       [128,1] f32
            c0 = btile[:, 2:4].bitcast(F32)   # C2 Einv F x0 [128,1] f32

            ps = [
                pspool.tile([128, CW], F32, tag=f"ps{n}", name=f"ps{n}")
                for n in range(NCH)
            ]
            ut = bpool.tile([128, BC], F16, tag="ut")

            w_cur = [None] * NCH
            w_prev = [None] * NCH

            # -- seed weights + bias first: transpose D12L & bias blocks,
            #    contiguous DVE copies out of PSUM --
            tpa = tpool.tile([128, CW], F16, tag="tst", name="tpa")
            nc.tensor.transpose(tpa[:, 0:128], cblk(0), ident[:])
            nc.tensor.transpose(tpa[:, 128:256], cblk(4), ident[:])
            nc.vector.tensor_copy(d12lt[:], tpa[:, 0:128])
            nc.vector.tensor_copy(btile[:], tpa[:, 128:132])

            def load_chunk(n):
                # 4 block transposes + one packed copy -> ut chunk n
                tpu = tpool.tile([128, CW], F16, tag="tst", name=f"tpu{n}")
                for k in range(4):
                    nc.tensor.transpose(
                        tpu[:, k * 128:(k + 1) * 128],
                        ustage[:, (4 * n + k) * 128:(4 * n + k + 1) * 128],
                        ident[:],
                    )
                sl = slice(n * CW, (n + 1) * CW)
                nc.vector.tensor_copy(ut[:, sl], tpu[:])

            def seed(n):
                sl = slice(n * CW, (n + 1) * CW)
                nc.tensor.matmul(ps[n][:], d12lt[:], ut[:, sl],
                                 start=True, stop=False)
                wt = wpool.tile([128, CW], F16, tag=f"w{n}", name=f"w{n}")
                nc.scalar.activation(wt[:], ps[n][:], TANH, bias=xcl)
                w_cur[n] = wt

            load_chunk(0)
            seed(0)
            # remaining pass/output weights while chunk 1 is in flight
            tpb = tpool.tile([128, CW], F16, tag="tst", name="tpb")
            nc.tensor.transpose(tpb[:, 0:128], cblk(1), ident[:])
            nc.tensor.transpose(tpb[:, 128:256], cblk(2), ident[:])
            nc.tensor.transpose(tpb[:, 256:384], cblk(3), ident[:])
            nc.vector.tensor_copy(ltr[:], tpb[:, 0:128])
            load_chunk(1)
            seed(1)
            nc.vector.tensor_copy(gut[:], tpb[:, 128:256])
            nc.vector.tensor_copy(gwt[:], tpb[:, 256:384])
            load_chunk(2)
            seed(2)
            load_chunk(3)
            seed(3)

            # -- delta-Jacobi passes: ps += Lhat @ (W_k - W_{k-1}) --
            for m in range(M_PASSES):
                last = m == M_PASSES - 1
                for n in range(NCH):
                    if m == 0:
                        dl = w_cur[n]  # W1 - 0
                    else:
                        dl = dpool.tile([128, CW], F16, tag=f"d{n}",
                                        name=f"d{n}")
                        nc.vector.tensor_sub(dl[:], w_cur[n][:], w_prev[n][:])
                    nc.tensor.matmul(ps[n][:], ltr[:], dl[:],
                                     start=False, stop=last)
                for n in range(NCH):
                    w_prev[n] = w_cur[n]
                    wt = wpool.tile([128, CW], F16, tag=f"w{n}", name=f"w{n}")
                    nc.scalar.activation(wt[:], ps[n][:], TANH, bias=xcl)
                    w_cur[n] = wt

            # -- per-chunk tail: po (reusing the chunk's freed ps bank) =
            #    Gu@Ut + Gw@W; yt = po + c0; store. c0-adds alternate
            #    DVE / ACT-Identity (same act table as tanh, no reload) --
            for n in range(NCH):
                sl = slice(n * CW, (n + 1) * CW)
                po = pspool.tile([128, CW], F32, tag=f"ps{n}", name=f"po{n}")
                nc.tensor.matmul(po[:], gut[:], ut[:, sl],
                                 start=True, stop=False)
                nc.tensor.matmul(po[:], gwt[:], w_cur[n][:],
                                 start=False, stop=True)
                yts = ypool.tile([128, CW], F16, tag=f"yt{n}", name=f"yt{n}")
                if n % 2 == 0:
                    nc.vector.tensor_scalar_add(yts[:], po[:], c0)
                else:
                    nc.scalar.activation(yts[:], po[:], IDENT, bias=c0)
                nc.sync.dma_start(y[:, sl], yts[:])
    nc.compile()
    return nc


def _derive_consts(X, Y, B2, C2, D21, D22, D12, x0):
    """Fold the contractive parameterization into kernel constants.
    Returns the [CROWS, 128] f16 constant region (matrix j's row m at
    DRAM row 5m + j; bias rows at 5p + 4)."""
    f = np.float32
    X = np.ascontiguousarray(X, f)
    H = (X.T @ X + EPS * np.eye(DIM_H, dtype=f)).astype(f)
    H11 = H[:DIM_X, :DIM_X]
    H21 = H[DIM_X:DIM_X + DIM_NL, :DIM_X]
    H22 = H[DIM_X:DIM_X + DIM_NL, DIM_X:DIM_X + DIM_NL]
    H31 = H[DIM_X + DIM_NL:, :DIM_X]
    H32 = H[DIM_X + DIM_NL:, DIM_X:DIM_X + DIM_NL]
    H33 = H[DIM_X + DIM_NL:, DIM_X + DIM_NL:]
    F = H31
    B1 = H32
    E = (0.5 * (H11 + ALPHA * H33 + Y - Y.T)).astype(f)
    Lam = (0.5 * np.diagonal(H22)).astype(f)
    D11 = (-np.tril(H22, k=-1)).astype(f)
    C1 = -H21

    Einv = np.linalg.inv(E).astype(f)
    x0v = np.asarray(x0, f)[0, 0, :]
    xc = (C1 @ x0v).astype(f)
    fx = (F @ x0v).astype(f)

    Lhat = (D11 / Lam[:, None]).astype(f)
    D12L = (np.asarray(D12, f) / Lam[:, None]).astype(f)
    CE = (np.asarray(C2, f) @ Einv).astype(f)
    Gu = (CE @ B2 + D22).astype(f)
    Gw = (CE @ B1 + D21).astype(f)
    xclam = (xc / Lam).astype(f)
    c0 = (CE @ fx).astype(f)

    h = np.float16
    V = np.zeros((128, 5, 128), h)
    V[:, 0] = D12L.astype(h)
    V[:, 1] = Lhat.astype(h)
    V[:, 2] = Gu.astype(h)
    V[:, 3] = Gw.astype(h)
    xb = xclam.view(np.uint32)
    cb = c0.view(np.uint32)
    V[0, 4] = (xb & 0xFFFF).astype(np.uint16).view(h)
    V[1, 4] = (xb >> 16).astype(np.uint16).view(h)
    V[2, 4] = (cb & 0xFFFF).astype(np.uint16).view(h)
    V[3, 4] = (cb >> 16).astype(np.uint16).view(h)
    return V.reshape(CROWS, 128)


def _make_in_maps(u_in, X, Y, B2, C2, D21, D22, D12, x0):
    cst = _derive_consts(X, Y, B2, C2, D21, D22, D12, x0)
    u16 = np.asarray(u_in, np.float32).reshape(B, DIM_IN).astype(np.float16)
    maps = []
    for i in range(N_CORES):
        uc = u16[i * BC:(i + 1) * BC]
        # per chunk: partition p holds rows {4p + r} = batch {128r + p}
        S = uc.reshape(NCH, 4, 128, DIM_IN).transpose(0, 2, 1, 3)
        ubuf = np.concatenate([cst, S.reshape(BC, DIM_IN)], axis=0)
        maps.append({"ub": np.ascontiguousarray(ubuf)})
    return maps


def kernel(u_in, X, Y, B2, C2, D21, D22, D12, x0):
    if "nc" not in _BUILT:
        _BUILT["nc"] = _build_nc()
    nc = _BUILT["nc"]
    in_maps = _make_in_maps(u_in, X, Y, B2, C2, D21, D22, D12, x0)
    res = run_bass_kernel_spmd(nc, in_maps, core_ids=list(range(N_CORES)))
    out = np.concatenate(
        [res.results[i]["y"].astype(np.float32).T for i in range(N_CORES)],
        axis=0,
    )
    return out.reshape(B, 1, DIM_OUT)
